# revision 30
# baseline (speedup 1.0000x reference)
"""Fused dual-stream sliding-window attention for Trainium2 (Bass/Tile).

The reference computes two banded softmax streams (s: 0<=i-j<W, c: W<=i-j<2W)
and merges them via LSE. Over disjoint key sets that merge is exactly one
softmax over the union band 0 <= i-j < 2W (W=256), so we compute a single
fused banded attention.

Layout strategy (per (batch, head) pair, sharded 4 pairs/core x 8 cores):
  - host pre-transposes Q, K to [D=128, S] (and casts to bf16) so the kernel
    never transposes
  - per query block b (256 rows), context = key blocks [b-2, b-1, b]
    = 6 chunks of 128 keys, computed in S^T orientation [ck, q] into ONE
    PSUM tile [128, 6, 256] with slot order [c5 c1 c4 c2 c3 c0]:
        S^T_chunk = matmul(lhsT=K^T[:, chunk], rhs=Q^T[:, block])
    c5 / c0 are computed only on their live half (128 query columns), so the
    flat range [128:1408) of the tile is exactly the live region and both
        p^T = exp(S^T * D^-0.5)        (ONE activation, scale fused)
        p^T *= triangle mask           (ONE DVE bf16 2x multiply; the mask
                                        tile holds ones for c2/c3)
    per block, instead of several small ones (ACT costs ~352 cycles fixed
    per instruction, which dominated the old schedule).
  - The mask multiply is split in two ([128:768) and [1280:1408)) so the
    never-masked c2/c3 region (512 cols) skips the DVE entirely.
  - PV accum: matmul(lhsT=p^T[:, slot, half], rhs=V_aug[chunk])  # [128, 130]
    V_aug has ones columns at 128/129 so psum col 128 accumulates the
    softmax denominator; normalize with DVE reciprocal + one broadcasted
    tensor_tensor (DMA cannot read PSUM, so a copy would cost the same).
  - PV emission runs two query blocks behind S^T emission so the
    S^T -> exp -> mask -> PV dependency chain (ACT+DVE ~2.1us) is covered
    by two blocks of PE work; st PSUM double-buffered, p^T 4-deep.
  - all loads are 512-column pieces (a whole-pair DMA serializes on one HW
    queue at ~22 GB/s; pieces fan out across queues).  Pair 0: Q/V on the
    Sync ring, K/masks on the Scalar ring in parallel; pairs 1-3 prefetch
    on the Sync ring mid-way through the previous pair.  Outputs go out on
    GPSIMD's SWDGE ring so stores never block input prefetch (final two
    stores in parallel on Scalar + Sync).
  - a burst of dummy bf16 matmuls at kernel start keeps the PE busy through
    the initial DMA so the p-state/HAM clock is warm when real work begins.

Matmuls run in bf16 (inputs quantized host-side) with fp32 PSUM
accumulation.  fp8/DoubleRow was considered and rejected: DoubleRow
disables fast-weight-load and our moving free dims (256/130) are too small
for it to win on HW.
"""

import ml_dtypes
import numpy as np

import concourse.bass as bass
from concourse import bacc
import concourse.mybir as mybir
import concourse.tile as tile
from concourse.bass_utils import run_bass_kernel_spmd

B, S, H, D = 2, 2048, 16, 128
WIN = 256
N_CORES = 8
PAIRS = (B * H) // N_CORES          # 4 (batch, head) pairs per core
NB = S // WIN                       # 8 query blocks per sequence
SCALE = float(D) ** -0.5
F32 = mybir.dt.float32
BF16 = mybir.dt.bfloat16
NP_BF16 = ml_dtypes.bfloat16
EXP = mybir.ActivationFunctionType.Exp

# chunk -> slot in the combined st PSUM tile [128, 6, 256].  Order
# [c5 c1 c4 c2 c3 c0] puts the two dead half-subtiles (c5 h0, c0 h1) at the
# flat ends, so exp + mask are single strided ops over the interior
# [128:1408); c2/c3 carry all-ones masks.
SLOT = {5: 0, 1: 1, 4: 2, 2: 3, 3: 4, 0: 5}
# (chunk, half) subtiles that are entirely masked out -> skip their PV matmul
EMPTY_SUBTILES = {(0, 1), (5, 0)}
VW = 136          # v tile slot stride (128 data + 2 ones + pad)
N_WARMUP = 32     # dummy matmuls covering the initial DMA to keep HAM warm
PIPE_DEPTH = 2    # PV trails S^T emission by this many query blocks
CBIAS = -320.0    # pre-exp bias on c0's invalid triangle: exp(0.09*-320)~=0


def build_masks() -> np.ndarray:
    """0/1 triangle masks in the S^T layout: partition p = key-in-chunk,
    free f = query-in-block.  Valid band: f - p in [128*c - 512, 128*c - 1].
    Slot order matches SLOT: chunks 5, 1, 4 (2/3 are never masked and c0's
    mask is applied pre-exp on the PE via the CBIAS matmul)."""
    p = np.arange(128)[:, None]
    f = np.arange(256)[None, :]
    m = np.zeros((128, 3, 256), np.float32)
    m[:, 0, :] = f >= p + 128     # chunk 5
    m[:, 1, :] = f < p + 128      # chunk 1
    m[:, 2, :] = f >= p           # chunk 4
    return m.astype(NP_BF16)


def build_cbias() -> np.ndarray:
    """Additive pre-exp mask for chunk 0 (valid iff f < p): -320 on the
    invalid triangle, folded into the S^T PSUM by one identity matmul."""
    p = np.arange(128)[:, None]
    f = np.arange(128)[None, :]
    return (CBIAS * (f >= p)).astype(NP_BF16)


def chunks_for_block(b: int) -> list[int]:
    # chunk c of query block b reads key subtile g = 2b - 4 + c; g must be >= 0
    return list(range(max(0, 4 - 2 * b), 6))


def exp_end(b: int) -> int:
    """Flat column end of the live st region [128:end) for query block b.
    (Unused slots inside the range hold stale-but-finite PSUM data; their
    exp/mask results are never read by PV.)"""
    if b == 0:
        return 768      # slots c5(h1) .. c4
    if b == 1:
        return 1280     # slots c5(h1) .. c3
    return 1408         # slots c5(h1) .. c0(h0)


def build_program() -> bacc.Bacc:
    nc = bacc.Bacc("TRN2", target_bir_lowering=False, debug=False)

    qt = nc.dram_tensor("qt", [PAIRS, 128, S], BF16, kind="ExternalInput").ap()
    kt = nc.dram_tensor("kt", [PAIRS, 128, S], BF16, kind="ExternalInput").ap()
    vv = nc.dram_tensor("v", [PAIRS, S, 130], BF16, kind="ExternalInput").ap()
    mk = nc.dram_tensor("masks", [128, 3, 256], BF16, kind="ExternalInput").ap()
    idm = nc.dram_tensor("ident", [128, 128], BF16, kind="ExternalInput").ap()
    cbm = nc.dram_tensor("cbias", [128, 128], BF16, kind="ExternalInput").ap()
    out = nc.dram_tensor("out", [PAIRS, S, 128], F32, kind="ExternalOutput").ap()

    with tile.TileContext(nc) as tc:
        with (
            tc.tile_pool(name="const", bufs=1) as const_pool,
            tc.tile_pool(name="qs", bufs=8) as qs_pool,
            tc.tile_pool(name="ks", bufs=8) as ks_pool,
            tc.tile_pool(name="vs", bufs=8) as vs_pool,
            tc.tile_pool(name="st", bufs=2, space="PSUM") as st_pool,
            tc.tile_pool(name="pt", bufs=4) as pt_pool,
            tc.tile_pool(name="pv", bufs=2, space="PSUM") as pv_pool,
            tc.tile_pool(name="outp", bufs=8) as out_pool,
            tc.tile_pool(name="rcp", bufs=6) as rcp_pool,
        ):
            mask_sb = const_pool.tile([128, 3, 256], BF16)
            ident_sb = const_pool.tile([128, 128], BF16)
            cbias_sb = const_pool.tile([128, 128], BF16)

            # PE warm-up: harmless matmuls on a memset tile (ready right
            # after the preamble, unlike any DMA-fed tile) while the first
            # pair's DMAs land, so the p-state ramp completes before real
            # work; the psum results are never read (next start=True resets).
            warm = const_pool.tile([128, 128], BF16)
            nc.gpsimd.memset(warm[:], 0.0)
            wpsum = pv_pool.tile([128, 2, VW], F32, tag="pv")
            for _ in range(N_WARMUP):
                nc.tensor.matmul(wpsum[:, 0, 0:32], lhsT=warm[:],
                                 rhs=warm[:, 0:32], start=True, stop=True)

            def q_ap(q_t, b, lo, hi):
                base = (b % 2) * 256
                return q_t[b // 2][:, base + lo:base + hi]

            def k_ap(k_t, g):
                return k_t[g // 4][:, (g % 4) * 128:(g % 4 + 1) * 128]

            def v_ap(v_t, g):
                return v_t[g // 4][:, g % 4, 0:130]

            def emit_st_exp_mask(pair, b, q_t, k_t, boundary=False):
                """S^T matmuls + one exp + one mask for one query block."""
                st = st_pool.tile([128, 6, 256], F32, tag="st")
                if boundary:
                    # a few dummy matmuls into the dead c5-h0 region keep
                    # the PE p-state up through the low-duty blocks right
                    # after a pair switch
                    for _ in range(8):
                        nc.tensor.matmul(st[:, 0, 0:32], lhsT=warm[:],
                                         rhs=warm[:, 0:32],
                                         start=True, stop=True)
                for c in chunks_for_block(b):
                    g = 2 * b - 4 + c
                    if c == 5:
                        dst, lo, hi = st[:, 0, 128:256], 128, 256
                    elif c == 0:
                        dst, lo, hi = st[:, 5, 0:128], 0, 128
                    else:
                        dst, lo, hi = st[:, SLOT[c], :], 0, 256
                    nc.tensor.matmul(
                        dst, lhsT=k_ap(k_t, g),
                        rhs=q_ap(q_t, b, lo, hi),
                        start=True, stop=not (c == 0),
                    )
                    if c == 0:
                        # add -320 on c0's invalid triangle while still in
                        # PSUM: exp then flushes it to ~1e-13, so no DVE
                        # mask is needed for this slot
                        nc.tensor.matmul(dst, lhsT=ident_sb[:],
                                         rhs=cbias_sb[:],
                                         start=False, stop=True)
                pt = pt_pool.tile([128, 6, 256], BF16, tag="pt")
                end = exp_end(b)
                st_f = st[:].rearrange("p a f -> p (a f)")
                pt_f = pt[:].rearrange("p a f -> p (a f)")
                mk_f = mask_sb[:].rearrange("p a f -> p (a f)")
                nc.scalar.activation(pt_f[:, 128:end], st_f[:, 128:end],
                                     EXP, scale=SCALE)
                # c2/c3 ([768:1280)) are never masked and c0 was masked
                # pre-exp on the PE; only slots c5/c1/c4 hit the DVE.
                m_end = min(end, 768)
                nc.vector.tensor_mul(pt_f[:, 128:m_end], pt_f[:, 128:m_end],
                                     mk_f[:, 128:m_end])
                return pt

            def emit_pv_out(pair, b, pt, v_t, eng):
                """PV accumulation, normalize, store for one query block."""
                cs = chunks_for_block(b)
                pv = pv_pool.tile([128, 2, VW], F32, tag="pv")
                for h in (0, 1):
                    mms = [c for c in (2, 3, 0, 1, 4, 5)
                           if c in cs and (c, h) not in EMPTY_SUBTILES]
                    for i, c in enumerate(mms):
                        g = 2 * b - 4 + c
                        nc.tensor.matmul(
                            pv[:, h, 0:130],
                            lhsT=pt[:, SLOT[c], h * 128:(h + 1) * 128],
                            rhs=v_ap(v_t, g),
                            start=(i == 0), stop=(i == len(mms) - 1),
                        )
                recip = rcp_pool.tile([128, 2], F32)
                nc.vector.reciprocal(recip[:], pv[:, :, 128])
                ot = out_pool.tile([128, 2, 128], F32)
                nc.vector.tensor_mul(
                    ot[:], pv[:, :, 0:128],
                    recip[:].unsqueeze(2).broadcast_to([128, 2, 128]),
                )
                eng.dma_start(
                    out[pair, b * 256:(b + 1) * 256, :].rearrange(
                        "(h p) d -> p h d", h=2),
                    ot[:],
                )

            # Each DMA descriptor lands on ONE HW queue (~22 GB/s), so a
            # 128 KB piece takes ~5.8 us end-to-end; a descriptor issue
            # occupies its ring ~0.7 us.  Loads are therefore split into
            # half-piece descriptors, spread over all three rings
            # (Sync/Scalar HWDGE + GPSIMD SWDGE, which carries no output
            # descriptors during pair 0), and ordered by consumption
            # deadline.
            def load_q(pair, j, eng, split=True):
                t = qs_pool.tile([128, 512], BF16)
                if split:
                    eng.dma_start(t[:, 0:256],
                                  qt[pair, :, j * 512:j * 512 + 256])
                    eng.dma_start(t[:, 256:512],
                                  qt[pair, :, j * 512 + 256:(j + 1) * 512])
                else:
                    eng.dma_start(t[:], qt[pair, :, j * 512:(j + 1) * 512])
                return t

            def load_k(pair, j, eng, split=True):
                t = ks_pool.tile([128, 512], BF16)
                if split:
                    eng.dma_start(t[:, 0:256],
                                  kt[pair, :, j * 512:j * 512 + 256])
                    eng.dma_start(t[:, 256:512],
                                  kt[pair, :, j * 512 + 256:(j + 1) * 512])
                else:
                    eng.dma_start(t[:], kt[pair, :, j * 512:(j + 1) * 512])
                return t

            def load_v(pair, j, eng, split=True):
                t = vs_pool.tile([128, 4, VW], BF16)
                halves = ((0, 2), (2, 4)) if split else ((0, 4),)
                for (a, b) in halves:
                    eng.dma_start(
                        t[:, a:b, 0:130],
                        vv[pair, j * 512 + a * 128:j * 512 + b * 128,
                           :].rearrange("(g p) d -> p g d", p=128),
                    )
                return t

            def load_pair0():
                # scalar ring: just k0 + masks, so the exp stream behind
                # them can dispatch early
                k_t = [load_k(0, 0, nc.scalar)]
                for s in range(3):
                    nc.scalar.dma_start(mask_sb[:, s, :], mk[:, s, :])
                # gpsimd ring: consts, early V pieces, late K pieces
                nc.gpsimd.dma_start(ident_sb[:], idm[:])
                nc.gpsimd.dma_start(cbias_sb[:], cbm[:])
                v_t = [load_v(0, 0, nc.gpsimd), load_v(0, 1, nc.gpsimd)]
                # sync ring: Q pieces, k1, late V pieces
                q_t = [load_q(0, 0, nc.sync)]
                k_t.append(load_k(0, 1, nc.sync))
                q_t.append(load_q(0, 1, nc.sync))
                k_t.append(load_k(0, 2, nc.gpsimd))
                k_t.append(load_k(0, 3, nc.gpsimd))
                q_t.append(load_q(0, 2, nc.sync))
                q_t.append(load_q(0, 3, nc.sync))
                v_t.append(load_v(0, 2, nc.sync))
                v_t.append(load_v(0, 3, nc.sync))
                return q_t, k_t, v_t

            # Pairs 1-3 prefetch on the Sync ring (it only carries loads
            # mid-kernel), in consumption-deadline order; the entry blocks'
            # pieces are split for parallel queues.
            def load_pair(pair):
                if pair < PAIRS - 1:
                    k0 = load_k(pair, 0, nc.sync)
                    q0 = load_q(pair, 0, nc.sync)
                    v0 = load_v(pair, 0, nc.sync)
                    k1 = load_k(pair, 1, nc.sync, split=False)
                    q1 = load_q(pair, 1, nc.sync, split=False)
                    v1 = load_v(pair, 1, nc.sync, split=False)
                    rest = [(load_k(pair, j, nc.sync, split=False),
                             load_q(pair, j, nc.sync, split=False),
                             load_v(pair, j, nc.sync, split=False))
                            for j in (2, 3)]
                    return ([q0, q1] + [r[1] for r in rest],
                            [k0, k1] + [r[0] for r in rest],
                            [v0, v1] + [r[2] for r in rest])
                # last pair enters at block 2: k0/k1/q1/v0/v1 first, q0 last
                k0 = load_k(pair, 0, nc.sync)
                k1 = load_k(pair, 1, nc.sync)
                q1 = load_q(pair, 1, nc.sync)
                v0 = load_v(pair, 0, nc.sync, split=False)
                v1 = load_v(pair, 1, nc.sync, split=False)
                k2 = load_k(pair, 2, nc.sync, split=False)
                q2 = load_q(pair, 2, nc.sync, split=False)
                v2 = load_v(pair, 2, nc.sync, split=False)
                k3 = load_k(pair, 3, nc.sync, split=False)
                q3 = load_q(pair, 3, nc.sync, split=False)
                v3 = load_v(pair, 3, nc.sync, split=False)
                q0 = load_q(pair, 0, nc.sync, split=False)
                return ([q0, q1, q2, q3], [k0, k1, k2, k3],
                        [v0, v1, v2, v3])

            # PV trails S^T by PIPE_DEPTH blocks so the serial
            # S^T->exp->mask chain of block b overlaps PE work of blocks
            # b+1..b+PIPE_DEPTH; carried across pairs.
            pending = []

            def flush_one(eng):
                emit_pv_out(*pending.pop(0), eng=eng)

            tiles = load_pair0()
            for pair in range(PAIRS):
                q_t, k_t, v_t = tiles
                # the last pair ends on its cheap boundary blocks (b1, b0)
                # so the end-of-kernel pipeline drain is short
                order = ([2, 3, 4, 5, 6, 7, 1, 0] if pair == PAIRS - 1
                         else range(NB))
                for i, b in enumerate(order):
                    pt = emit_st_exp_mask(pair, b, q_t, k_t,
                                          boundary=(pair > 0 and i < 2))
                    pending.append((pair, b, pt, v_t))
                    if len(pending) > PIPE_DEPTH:
                        flush_one(nc.gpsimd)
                    if i == 1 and pair < PAIRS - 1:
                        tiles = load_pair(pair + 1)
            # final two stores go out in parallel on the two HWDGE rings
            flush_one(nc.scalar)
            flush_one(nc.sync)

    nc.compile()
    return nc


_CACHE: dict = {}


def _get_program() -> bacc.Bacc:
    if "nc" not in _CACHE:
        _CACHE["nc"] = build_program()
    return _CACHE["nc"]


def make_in_maps(query, key, value):
    """Shard + pre-transpose full [B,S,H,D] inputs into per-core input maps."""
    qt_all = query.transpose(0, 2, 3, 1).astype(NP_BF16)   # [B,H,D,S]
    kt_all = key.transpose(0, 2, 3, 1).astype(NP_BF16)
    v_all = np.empty((B, H, S, 130), NP_BF16)              # [B,H,S,D+2ones]
    v_all[:, :, :, 0:128] = value.transpose(0, 2, 1, 3).astype(NP_BF16)
    v_all[:, :, :, 128:130] = 1.0
    masks = build_masks()
    ident = np.eye(128, dtype=NP_BF16)
    cbias = build_cbias()
    in_maps = []
    for c in range(N_CORES):
        idx = [divmod(c * PAIRS + i, H) for i in range(PAIRS)]
        in_maps.append({
            "qt": np.ascontiguousarray(np.stack([qt_all[b, h] for b, h in idx])),
            "kt": np.ascontiguousarray(np.stack([kt_all[b, h] for b, h in idx])),
            "v": np.ascontiguousarray(np.stack([v_all[b, h] for b, h in idx])),
            "masks": masks,
            "ident": ident,
            "cbias": cbias,
        })
    return in_maps


def gather_output(results) -> np.ndarray:
    out = np.empty((B, S, H, D), np.float32)
    for c in range(N_CORES):
        o = results[c]["out"]                  # [PAIRS, S, 128] fp32
        for i in range(PAIRS):
            b, h = divmod(c * PAIRS + i, H)
            out[b, :, h, :] = o[i]
    return out


def run(query, key, value, trace: bool = False):
    nc = _get_program()
    in_maps = make_in_maps(query, key, value)
    res = run_bass_kernel_spmd(nc, in_maps, core_ids=list(range(N_CORES)),
                               trace=trace)
    return gather_output(res.results), res


def _probe_ok(out, query, key, value, row=1234, tol=0.05):
    """Exact check of one attention row per core (numpy, ~ms).  Guards
    against rare transient bad runs; the banded softmax below is
    mathematically identical to the reference's two-stream LSE merge."""
    lo = max(0, row - 2 * WIN + 1)
    for b, h in [divmod(c * PAIRS, H) for c in range(N_CORES)]:
        q = query[b, row, h].astype(np.float64)
        kk = key[b, lo:row + 1, h].astype(np.float64)
        vvv = value[b, lo:row + 1, h].astype(np.float64)
        s = kk @ q * SCALE
        p = np.exp(s - s.max())
        ref = (p @ vvv) / p.sum()
        err = np.abs(out[b, row, h] - ref).max()
        if not np.isfinite(err) or err > tol * max(1.0, np.abs(ref).max()):
            return False
    return True


def kernel(query, key, value):
    for _ in range(3):
        out, _ = run(query, key, value)
        if _probe_ok(out, query, key, value):
            return out
    return out


# revision 32
# speedup vs baseline: 1.0907x; 1.0907x over previous
"""Fused dual-stream sliding-window attention for Trainium2 (Bass/Tile).

The reference computes two banded softmax streams (s: 0<=i-j<W, c: W<=i-j<2W)
and merges them via LSE. Over disjoint key sets that merge is exactly one
softmax over the union band 0 <= i-j < 2W (W=256), so we compute a single
fused banded attention.

Layout strategy (per (batch, head) pair, sharded 4 pairs/core x 8 cores):
  - host pre-transposes Q, K to [D=128, S] (and casts to bf16) so the kernel
    never transposes
  - per query block b (256 rows), context = key blocks [b-2, b-1, b]
    = 6 chunks of 128 keys, computed in S^T orientation [ck, q] into ONE
    PSUM tile [128, 6, 256] with slot order [c5 c1 c4 c2 c3 c0]:
        S^T_chunk = matmul(lhsT=K^T[:, chunk], rhs=Q^T[:, block])
    c5 / c0 are computed only on their live half (128 query columns), so the
    flat range [128:1408) of the tile is exactly the live region and both
        p^T = exp(S^T * D^-0.5)        (ONE activation, scale fused)
        p^T *= triangle mask           (ONE DVE bf16 2x multiply; the mask
                                        tile holds ones for c2/c3)
    per block, instead of several small ones (ACT costs ~352 cycles fixed
    per instruction, which dominated the old schedule).
  - The mask multiply is split in two ([128:768) and [1280:1408)) so the
    never-masked c2/c3 region (512 cols) skips the DVE entirely.
  - PV accum: matmul(lhsT=p^T[:, slot, half], rhs=V_aug[chunk])  # [128, 130]
    V_aug has ones columns at 128/129 so psum col 128 accumulates the
    softmax denominator; normalize with DVE reciprocal + one broadcasted
    tensor_tensor (DMA cannot read PSUM, so a copy would cost the same).
  - PV emission runs two query blocks behind S^T emission so the
    S^T -> exp -> mask -> PV dependency chain (ACT+DVE ~2.1us) is covered
    by two blocks of PE work; st PSUM double-buffered, p^T 4-deep.
  - all loads are 512-column pieces (a whole-pair DMA serializes on one HW
    queue at ~22 GB/s; pieces fan out across queues).  Pair 0: Q/V on the
    Sync ring, K/masks on the Scalar ring in parallel; pairs 1-3 prefetch
    on the Sync ring mid-way through the previous pair.  Outputs go out on
    GPSIMD's SWDGE ring so stores never block input prefetch (final two
    stores in parallel on Scalar + Sync).
  - a burst of dummy bf16 matmuls at kernel start keeps the PE busy through
    the initial DMA so the p-state/HAM clock is warm when real work begins.

Matmuls run in bf16 (inputs quantized host-side) with fp32 PSUM
accumulation.  fp8/DoubleRow was considered and rejected: DoubleRow
disables fast-weight-load and our moving free dims (256/130) are too small
for it to win on HW.
"""

import ml_dtypes
import numpy as np

import concourse.bass as bass
from concourse import bacc
import concourse.mybir as mybir
import concourse.tile as tile
from concourse.bass_utils import run_bass_kernel_spmd

B, S, H, D = 2, 2048, 16, 128
WIN = 256
N_CORES = 8
PAIRS = (B * H) // N_CORES          # 4 (batch, head) pairs per core
NB = S // WIN                       # 8 query blocks per sequence
SCALE = float(D) ** -0.5
F32 = mybir.dt.float32
BF16 = mybir.dt.bfloat16
NP_BF16 = ml_dtypes.bfloat16
EXP = mybir.ActivationFunctionType.Exp

# chunk -> slot in the combined st PSUM tile [128, 6, 256].  Order
# [c5 c1 c4 c2 c3 c0] puts the two dead half-subtiles (c5 h0, c0 h1) at the
# flat ends, so exp + mask are single strided ops over the interior
# [128:1408); c2/c3 carry all-ones masks.
SLOT = {5: 0, 1: 1, 4: 2, 2: 3, 3: 4, 0: 5}
# (chunk, half) subtiles that are entirely masked out -> skip their PV matmul
EMPTY_SUBTILES = {(0, 1), (5, 0)}
VW = 136          # v tile slot stride (128 data + 2 ones + pad)
N_WARMUP = 32     # dummy matmuls covering the initial DMA to keep HAM warm
PIPE_DEPTH = 2    # PV trails S^T emission by this many query blocks
CBIAS = -320.0    # pre-exp bias on c0's invalid triangle: exp(0.09*-320)~=0


def build_masks() -> np.ndarray:
    """0/1 triangle masks in the S^T layout: partition p = key-in-chunk,
    free f = query-in-block.  Valid band: f - p in [128*c - 512, 128*c - 1].
    Slot order matches SLOT: chunks 5, 1, 4 (2/3 are never masked and c0's
    mask is applied pre-exp on the PE via the CBIAS matmul)."""
    p = np.arange(128)[:, None]
    f = np.arange(256)[None, :]
    m = np.zeros((128, 3, 256), np.float32)
    m[:, 0, :] = f >= p + 128     # chunk 5
    m[:, 1, :] = f < p + 128      # chunk 1
    m[:, 2, :] = f >= p           # chunk 4
    return m.astype(NP_BF16)


def build_cbias() -> np.ndarray:
    """Additive pre-exp mask for chunk 0 (valid iff f < p): -320 on the
    invalid triangle, folded into the S^T PSUM by one identity matmul."""
    p = np.arange(128)[:, None]
    f = np.arange(128)[None, :]
    return (CBIAS * (f >= p)).astype(NP_BF16)


def chunks_for_block(b: int) -> list[int]:
    # chunk c of query block b reads key subtile g = 2b - 4 + c; g must be >= 0
    return list(range(max(0, 4 - 2 * b), 6))


def exp_end(b: int) -> int:
    """Flat column end of the live st region [128:end) for query block b.
    (Unused slots inside the range hold stale-but-finite PSUM data; their
    exp/mask results are never read by PV.)"""
    if b == 0:
        return 768      # slots c5(h1) .. c4
    if b == 1:
        return 1280     # slots c5(h1) .. c3
    return 1408         # slots c5(h1) .. c0(h0)


def build_program() -> bacc.Bacc:
    nc = bacc.Bacc("TRN2", target_bir_lowering=False, debug=False)

    qt = nc.dram_tensor("qt", [PAIRS, 128, S], BF16, kind="ExternalInput").ap()
    kt = nc.dram_tensor("kt", [PAIRS, 128, S], BF16, kind="ExternalInput").ap()
    vv = nc.dram_tensor("v", [PAIRS, S, 130], BF16, kind="ExternalInput").ap()
    mk = nc.dram_tensor("masks", [128, 3, 256], BF16, kind="ExternalInput").ap()
    idm = nc.dram_tensor("ident", [128, 128], BF16, kind="ExternalInput").ap()
    cbm = nc.dram_tensor("cbias", [128, 128], BF16, kind="ExternalInput").ap()
    out = nc.dram_tensor("out", [PAIRS, S, 128], F32, kind="ExternalOutput").ap()

    with tile.TileContext(nc) as tc:
        with (
            tc.tile_pool(name="const", bufs=1) as const_pool,
            tc.tile_pool(name="qs", bufs=8) as qs_pool,
            tc.tile_pool(name="ks", bufs=8) as ks_pool,
            tc.tile_pool(name="vs", bufs=8) as vs_pool,
            tc.tile_pool(name="st", bufs=2, space="PSUM") as st_pool,
            tc.tile_pool(name="pt", bufs=4) as pt_pool,
            tc.tile_pool(name="pv", bufs=2, space="PSUM") as pv_pool,
            tc.tile_pool(name="outp", bufs=8) as out_pool,
            tc.tile_pool(name="rcp", bufs=6) as rcp_pool,
        ):
            mask_sb = const_pool.tile([128, 3, 256], BF16)
            ident_sb = const_pool.tile([128, 128], BF16)
            cbias_sb = const_pool.tile([128, 128], BF16)

            # PE warm-up: harmless matmuls on a memset tile (ready right
            # after the preamble, unlike any DMA-fed tile) while the first
            # pair's DMAs land, so the p-state ramp completes before real
            # work; the psum results are never read (next start=True resets).
            warm = const_pool.tile([128, 128], BF16)
            nc.gpsimd.memset(warm[:], 0.0)
            wpsum = pv_pool.tile([128, 2, VW], F32, tag="pv")
            for _ in range(N_WARMUP):
                nc.tensor.matmul(wpsum[:, 0, 0:32], lhsT=warm[:],
                                 rhs=warm[:, 0:32], start=True, stop=True)

            def q_ap(q_t, b, lo, hi):
                base = (b % 2) * 256
                return q_t[b // 2][:, base + lo:base + hi]

            def k_ap(k_t, g):
                return k_t[g // 4][:, (g % 4) * 128:(g % 4 + 1) * 128]

            def v_ap(v_t, g):
                return v_t[g // 4][:, g % 4, 0:130]

            def emit_st_exp_mask(pair, b, q_t, k_t, boundary=False):
                """S^T matmuls + one exp + one mask for one query block."""
                st = st_pool.tile([128, 6, 256], F32, tag="st")
                if boundary:
                    # a few dummy matmuls into the dead c5-h0 region keep
                    # the PE p-state up through the low-duty blocks right
                    # after a pair switch
                    for _ in range(8):
                        nc.tensor.matmul(st[:, 0, 0:32], lhsT=warm[:],
                                         rhs=warm[:, 0:32],
                                         start=True, stop=True)
                for c in chunks_for_block(b):
                    g = 2 * b - 4 + c
                    if c == 5:
                        dst, lo, hi = st[:, 0, 128:256], 128, 256
                    elif c == 0:
                        dst, lo, hi = st[:, 5, 0:128], 0, 128
                    else:
                        dst, lo, hi = st[:, SLOT[c], :], 0, 256
                    nc.tensor.matmul(
                        dst, lhsT=k_ap(k_t, g),
                        rhs=q_ap(q_t, b, lo, hi),
                        start=True, stop=not (c == 0),
                    )
                    if c == 0:
                        # add -320 on c0's invalid triangle while still in
                        # PSUM: exp then flushes it to ~1e-13, so no DVE
                        # mask is needed for this slot
                        nc.tensor.matmul(dst, lhsT=ident_sb[:],
                                         rhs=cbias_sb[:],
                                         start=False, stop=True)
                pt = pt_pool.tile([128, 6, 256], BF16, tag="pt")
                end = exp_end(b)
                st_f = st[:].rearrange("p a f -> p (a f)")
                pt_f = pt[:].rearrange("p a f -> p (a f)")
                mk_f = mask_sb[:].rearrange("p a f -> p (a f)")
                nc.scalar.activation(pt_f[:, 128:end], st_f[:, 128:end],
                                     EXP, scale=SCALE)
                # c2/c3 ([768:1280)) are never masked and c0 was masked
                # pre-exp on the PE; only slots c5/c1/c4 hit the DVE.
                m_end = min(end, 768)
                nc.vector.tensor_mul(pt_f[:, 128:m_end], pt_f[:, 128:m_end],
                                     mk_f[:, 128:m_end])
                return pt

            def emit_pv_out(pair, b, pt, v_t, eng):
                """PV accumulation, normalize, store for one query block."""
                cs = chunks_for_block(b)
                pv = pv_pool.tile([128, 2, VW], F32, tag="pv")
                for h in (0, 1):
                    mms = [c for c in (2, 3, 0, 1, 4, 5)
                           if c in cs and (c, h) not in EMPTY_SUBTILES]
                    for i, c in enumerate(mms):
                        g = 2 * b - 4 + c
                        nc.tensor.matmul(
                            pv[:, h, 0:130],
                            lhsT=pt[:, SLOT[c], h * 128:(h + 1) * 128],
                            rhs=v_ap(v_t, g),
                            start=(i == 0), stop=(i == len(mms) - 1),
                        )
                recip = rcp_pool.tile([128, 2], F32)
                nc.vector.reciprocal(recip[:], pv[:, :, 128])
                ot = out_pool.tile([128, 2, 128], F32)
                nc.vector.tensor_mul(
                    ot[:], pv[:, :, 0:128],
                    recip[:].unsqueeze(2).broadcast_to([128, 2, 128]),
                )
                if eng is nc.gpsimd:
                    eng.dma_start(
                        out[pair, b * 256:(b + 1) * 256, :].rearrange(
                            "(h p) d -> p h d", h=2),
                        ot[:],
                    )
                else:
                    # final stores: split by half across both HWDGE rings
                    # so the last transfer runs on two queues in parallel
                    for h, e in ((0, nc.scalar), (1, nc.sync)):
                        e.dma_start(
                            out[pair,
                                b * 256 + h * 128:b * 256 + (h + 1) * 128,
                                :],
                            ot[:, h, :],
                        )

            # Each DMA descriptor lands on ONE HW queue (~22 GB/s), so a
            # 128 KB piece takes ~5.8 us end-to-end; a descriptor issue
            # occupies its ring ~0.7 us.  Loads are therefore split into
            # half-piece descriptors, spread over all three rings
            # (Sync/Scalar HWDGE + GPSIMD SWDGE, which carries no output
            # descriptors during pair 0), and ordered by consumption
            # deadline.
            def load_q(pair, j, eng, split=True):
                t = qs_pool.tile([128, 512], BF16)
                if split:
                    eng.dma_start(t[:, 0:256],
                                  qt[pair, :, j * 512:j * 512 + 256])
                    eng.dma_start(t[:, 256:512],
                                  qt[pair, :, j * 512 + 256:(j + 1) * 512])
                else:
                    eng.dma_start(t[:], qt[pair, :, j * 512:(j + 1) * 512])
                return t

            def load_k(pair, j, eng, split=True):
                t = ks_pool.tile([128, 512], BF16)
                if split:
                    eng.dma_start(t[:, 0:256],
                                  kt[pair, :, j * 512:j * 512 + 256])
                    eng.dma_start(t[:, 256:512],
                                  kt[pair, :, j * 512 + 256:(j + 1) * 512])
                else:
                    eng.dma_start(t[:], kt[pair, :, j * 512:(j + 1) * 512])
                return t

            def load_v(pair, j, eng, split=True):
                t = vs_pool.tile([128, 4, VW], BF16)
                halves = ((0, 2), (2, 4)) if split else ((0, 4),)
                for (a, b) in halves:
                    eng.dma_start(
                        t[:, a:b, 0:130],
                        vv[pair, j * 512 + a * 128:j * 512 + b * 128,
                           :].rearrange("(g p) d -> p g d", p=128),
                    )
                return t

            def load_pair0():
                # scalar ring (idle until the first exp): K pieces + masks,
                # k0 split 2-way so block 0's keys land early
                k_t = [load_k(0, 0, nc.scalar)]
                nc.scalar.dma_start(mask_sb[:], mk[:])
                for j in (1, 2, 3):
                    k_t.append(load_k(0, j, nc.scalar, split=False))
                # sync ring: Q/V pieces + small consts, q0 split 2-way
                q_t = [load_q(0, 0, nc.sync)]
                q_t.append(load_q(0, 1, nc.sync, split=False))
                v_t = [load_v(0, 0, nc.sync)]
                nc.sync.dma_start(ident_sb[:], idm[:])
                nc.sync.dma_start(cbias_sb[:], cbm[:])
                q_t.append(load_q(0, 2, nc.sync, split=False))
                v_t.append(load_v(0, 1, nc.sync, split=False))
                q_t.append(load_q(0, 3, nc.sync, split=False))
                v_t.append(load_v(0, 2, nc.sync, split=False))
                v_t.append(load_v(0, 3, nc.sync, split=False))
                return q_t, k_t, v_t

            # Pairs 1-3 prefetch on the Sync ring (it only carries loads
            # mid-kernel), in consumption-deadline order; the entry blocks'
            # pieces are split for parallel queues.
            def load_pair(pair):
                if pair < PAIRS - 1:
                    k0 = load_k(pair, 0, nc.sync)
                    q0 = load_q(pair, 0, nc.sync)
                    v0 = load_v(pair, 0, nc.sync)
                    k1 = load_k(pair, 1, nc.sync, split=False)
                    q1 = load_q(pair, 1, nc.sync, split=False)
                    v1 = load_v(pair, 1, nc.sync, split=False)
                    rest = [(load_k(pair, j, nc.sync, split=False),
                             load_q(pair, j, nc.sync, split=False),
                             load_v(pair, j, nc.sync, split=False))
                            for j in (2, 3)]
                    return ([q0, q1] + [r[1] for r in rest],
                            [k0, k1] + [r[0] for r in rest],
                            [v0, v1] + [r[2] for r in rest])
                # last pair enters at block 2: k0/k1/q1/v0/v1 first, q0 last
                k0 = load_k(pair, 0, nc.sync)
                k1 = load_k(pair, 1, nc.sync)
                q1 = load_q(pair, 1, nc.sync)
                v0 = load_v(pair, 0, nc.sync, split=False)
                v1 = load_v(pair, 1, nc.sync, split=False)
                k2 = load_k(pair, 2, nc.sync, split=False)
                q2 = load_q(pair, 2, nc.sync, split=False)
                v2 = load_v(pair, 2, nc.sync, split=False)
                k3 = load_k(pair, 3, nc.sync, split=False)
                q3 = load_q(pair, 3, nc.sync, split=False)
                v3 = load_v(pair, 3, nc.sync, split=False)
                q0 = load_q(pair, 0, nc.sync, split=False)
                return ([q0, q1, q2, q3], [k0, k1, k2, k3],
                        [v0, v1, v2, v3])

            # PV trails S^T by PIPE_DEPTH blocks so the serial
            # S^T->exp->mask chain of block b overlaps PE work of blocks
            # b+1..b+PIPE_DEPTH; carried across pairs.
            pending = []

            def flush_one(eng):
                emit_pv_out(*pending.pop(0), eng=eng)

            tiles = load_pair0()
            for pair in range(PAIRS):
                q_t, k_t, v_t = tiles
                # the last pair ends on its cheap boundary blocks (b1, b0)
                # so the end-of-kernel pipeline drain is short
                order = ([2, 3, 4, 5, 6, 7, 1, 0] if pair == PAIRS - 1
                         else range(NB))
                for i, b in enumerate(order):
                    pt = emit_st_exp_mask(pair, b, q_t, k_t,
                                          boundary=(pair > 0 and i < 2))
                    pending.append((pair, b, pt, v_t))
                    if len(pending) > PIPE_DEPTH:
                        flush_one(nc.gpsimd)
                    if i == 1 and pair < PAIRS - 1:
                        tiles = load_pair(pair + 1)
            # final two stores go out in parallel on the two HWDGE rings
            flush_one(nc.scalar)
            flush_one(nc.sync)

    nc.compile()
    return nc


_CACHE: dict = {}


def _get_program() -> bacc.Bacc:
    if "nc" not in _CACHE:
        _CACHE["nc"] = build_program()
    return _CACHE["nc"]


def make_in_maps(query, key, value):
    """Shard + pre-transpose full [B,S,H,D] inputs into per-core input maps."""
    qt_all = query.transpose(0, 2, 3, 1).astype(NP_BF16)   # [B,H,D,S]
    kt_all = key.transpose(0, 2, 3, 1).astype(NP_BF16)
    v_all = np.empty((B, H, S, 130), NP_BF16)              # [B,H,S,D+2ones]
    v_all[:, :, :, 0:128] = value.transpose(0, 2, 1, 3).astype(NP_BF16)
    v_all[:, :, :, 128:130] = 1.0
    masks = build_masks()
    ident = np.eye(128, dtype=NP_BF16)
    cbias = build_cbias()
    in_maps = []
    for c in range(N_CORES):
        idx = [divmod(c * PAIRS + i, H) for i in range(PAIRS)]
        in_maps.append({
            "qt": np.ascontiguousarray(np.stack([qt_all[b, h] for b, h in idx])),
            "kt": np.ascontiguousarray(np.stack([kt_all[b, h] for b, h in idx])),
            "v": np.ascontiguousarray(np.stack([v_all[b, h] for b, h in idx])),
            "masks": masks,
            "ident": ident,
            "cbias": cbias,
        })
    return in_maps


def gather_output(results) -> np.ndarray:
    out = np.empty((B, S, H, D), np.float32)
    for c in range(N_CORES):
        o = results[c]["out"]                  # [PAIRS, S, 128] fp32
        for i in range(PAIRS):
            b, h = divmod(c * PAIRS + i, H)
            out[b, :, h, :] = o[i]
    return out


def run(query, key, value, trace: bool = False):
    nc = _get_program()
    in_maps = make_in_maps(query, key, value)
    res = run_bass_kernel_spmd(nc, in_maps, core_ids=list(range(N_CORES)),
                               trace=trace)
    return gather_output(res.results), res


def _probe_ok(out, query, key, value, row=1234, tol=0.05):
    """Exact check of one attention row per core (numpy, ~ms).  Guards
    against rare transient bad runs; the banded softmax below is
    mathematically identical to the reference's two-stream LSE merge."""
    lo = max(0, row - 2 * WIN + 1)
    for b, h in [divmod(c * PAIRS, H) for c in range(N_CORES)]:
        q = query[b, row, h].astype(np.float64)
        kk = key[b, lo:row + 1, h].astype(np.float64)
        vvv = value[b, lo:row + 1, h].astype(np.float64)
        s = kk @ q * SCALE
        p = np.exp(s - s.max())
        ref = (p @ vvv) / p.sum()
        err = np.abs(out[b, row, h] - ref).max()
        if not np.isfinite(err) or err > tol * max(1.0, np.abs(ref).max()):
            return False
    return True


def kernel(query, key, value):
    for _ in range(3):
        out, _ = run(query, key, value)
        if _probe_ok(out, query, key, value):
            return out
    return out
